# revision 4
# baseline (speedup 1.0000x reference)
"""COGConv2d Trainium2 kernel (8 NeuronCores, Bass/Tile).

Reference computation (per sample b):
  pooled = mean_{h,w} x[b]                               [C]
  h      = relu(fc1_w @ pooled)                          [C]
  kern   = fc2_w @ h + fc2_b                             [CH*C], u = c*CH + t
  cw[o,c,i,j]   = sum_t kern[c*CH+t] * cog[o,t,i,j]
  dynw[o,c,i,j] = sigmoid(cw) * weight[o,c,i,j]
  y[b]   = conv2d(x[b], dynw, pad=1)                     [O,H,W]

Sharding: data-parallel over batch B=32 across 8 cores (4 samples/core);
the small static params are replicated to every core.

Per core, the conv runs as 9-tap shifted matmuls accumulating in PSUM
([dynw tap slice].T @ [shifted x window], contraction over channels),
ct-major so each row block only needs dynw for one ctile at a time.
x is zero-padded to 58x58 on the host so every tap window is a simple
AP slice of one SBUF tile.

The whole datapath is bf16 (PE runs bf16 at the same 1 cycle/row as
fp32r, but DMA bytes halve; y is stored bf16 and widened on the host).
The per-sample weight synthesis runs on-chip: all 16 fc2 matmuls
accumulate into one PSUM bank (one column per output chunk) and a single
DVE add applies the bias, with a host-side fc2 column permutation chosen
so a PE transpose of each [128, CH] kern slice directly yields the
[CH, 128] stationary for a compact K=CH cw matmul against cog4[t, :].
The synthesis for sample b+1 is pipelined into sample b's conv. Startup
DMAs are issued in critical-path order (x0, fc1, fc2, ident, cog4, w)
since the DMA engine drains its queue serially, and a dummy matmul right
after init starts the PE p-state ramp so real matmuls run at full clock.
The final row block is split 7+1 so the last store's copy+DMA tail is
minimal.
"""

import numpy as np
import ml_dtypes

import concourse.bacc as bacc
import concourse.mybir as mybir
import concourse.tile as tile
from concourse.bass_utils import run_bass_kernel_spmd

F32 = mybir.dt.float32
BF16 = mybir.dt.bfloat16
AF = mybir.ActivationFunctionType

N_CORES = 8
B, C, O, KS, H, W, CH = 32, 256, 256, 3, 56, 56, 4
BL = B // N_CORES            # samples per core
HW = H * W                   # 3136
HP, WP = H + 2, W + 2        # host-padded spatial (58x58)
XPADN = HP * WP + 4          # padded map + 4 spare cols (3368)
IJO = KS * KS * O            # 2304; dyn-weight free index = (i*3+j)*O + o
CT = C // 128                # contraction tiles (2)
OT = O // 128                # output-channel tiles (2)
RROWS = 8                    # output rows per conv matmul block
RB = H // RROWS              # row blocks (7)
NCONV = RROWS * W            # conv matmul moving size (448)
HWINV = 1.0 / HW
UQ = CH * C // 128           # fc2 output chunks (8)
NXQ = 2                      # x load split for load/reduce overlap
CW_CHUNKS = [(o, min(512, IJO - o)) for o in range(0, IJO, 512)]

_CACHE = {}


def _emit_synth(nc, b, ctx_tiles, act_assist=False):
    """Weight synthesis part 1 for sample b: pooled -> fc1 -> fc2 -> kern.

    kern layout: [128 partitions (u), q cols] where column q = ct*CH + t
    holds, at partition u, the fc2 output for (c = ct*128 + u, t) -- the
    host permutes fc2's columns to produce this order. All fc2 matmuls
    accumulate into single-bank PSUM columns (no per-chunk ACT ping-pong);
    one DVE add applies the bias. Returns kget(ct) -> [128, CH] slice.
    """
    (pool, psum_fc, xsb, fc1_sb, fc2_sb, fc2b_sb) = ctx_tiles

    pooled = [
        pool.tile([128, 1], F32, name=f"pooled{b}_{ct}", tag=f"pooled{ct}", bufs=2)
        for ct in range(CT)
    ]
    xq = XPADN // NXQ
    for ct in range(CT):
        rp = pool.tile([128, NXQ], F32, name=f"rp{b}_{ct}", tag=f"rp{ct}", bufs=2)
        for q in range(NXQ):
            if act_assist and ct == 1:
                # head: DVE is the pooled bottleneck; ct1 partials on ACT
                scr = pool.tile([128, xq], BF16, name=f"rs{b}_{q}", tag="rs", bufs=2)
                nc.scalar.activation(
                    scr[:], xsb[b][ct][:, q * xq : (q + 1) * xq],
                    AF.Copy, accum_out=rp[:, q : q + 1],
                )
            else:
                nc.vector.tensor_reduce(
                    out=rp[:, q : q + 1],
                    in_=xsb[b][ct][:, q * xq : (q + 1) * xq],
                    axis=mybir.AxisListType.X, op=mybir.AluOpType.add,
                )
        nc.vector.tensor_reduce(
            out=pooled[ct][:], in_=rp[:], axis=mybir.AxisListType.X,
            op=mybir.AluOpType.add,
        )

    pfcA = psum_fc.tile([128, CT], F32, name=f"pfcA{b}", tag="pfcA", bufs=1)
    for it in range(CT):
        for jt in range(CT):
            nc.tensor.matmul(
                pfcA[:, it : it + 1],
                fc1_sb[jt][:, it * 128 : (it + 1) * 128], pooled[jt][:],
                start=(jt == 0), stop=(jt == CT - 1), skip_group_check=True,
            )
    hv = pool.tile([128, CT], BF16, name=f"hv{b}", tag="hv", bufs=2)
    nc.scalar.activation(hv[:], pfcA[:], AF.Relu)

    pfcB = psum_fc.tile([128, UQ], F32, name=f"pfcB{b}", tag="pfcB", bufs=1)
    kern = pool.tile([128, UQ], BF16, name=f"kern_sb{b}", tag="kern_sb", bufs=2)
    for q in range(UQ):
        for jt in range(CT):
            nc.tensor.matmul(
                pfcB[:, q : q + 1],
                fc2_sb[jt][:, q * 128 : (q + 1) * 128], hv[:, jt : jt + 1],
                start=(jt == 0), stop=(jt == CT - 1), skip_group_check=True,
            )
        if q == CH - 1 or q == UQ - 1:
            lo = 0 if q == CH - 1 else CH
            nc.vector.tensor_add(
                kern[:, lo : q + 1], pfcB[:, lo : q + 1], fc2b_sb[:, lo : q + 1]
            )

    return lambda ct: kern[:, ct * CH : (ct + 1) * CH]


def _emit_dynw(nc, b, kget, ctx_tiles):
    """Part 2: transpose kern slice -> K=CH cw matmuls -> sigmoid -> *w.

    cw[c, n] = sum_t kern2[t, c] * cog4[t, n] with kern2 = kget(ct).T via
    the PE transpose path (output bounced through PSUM then SBUF).
    """
    (pool, psum_cw, psum_kt, cog4_sb, ident_sb, w_sb) = ctx_tiles
    dynw = [
        pool.tile([128, IJO], BF16, name=f"dynw{b}_{ct}", tag=f"dynw{ct}", bufs=2)
        for ct in range(CT)
    ]
    for ct in range(CT):
        ktr = psum_kt.tile([CH, 128], BF16, name=f"ktr{b}_{ct}", tag="ktr", bufs=1)
        nc.tensor.transpose(ktr[:], kget(ct), ident_sb[:])
        kern2 = pool.tile([CH, 128], BF16, name=f"k2_{b}_{ct}", tag=f"k2_{ct}", bufs=2)
        nc.vector.tensor_copy(kern2[:], ktr[:])
        for off, ln in CW_CHUNKS:
            pcw = psum_cw.tile(
                [128, 512], F32, name=f"pcw{b}_{ct}_{off}", tag="pcw", bufs=2
            )
            nc.tensor.matmul(
                pcw[:, :ln], kern2[:], cog4_sb[:, off : off + ln],
                start=True, stop=True,
            )
            dslice = dynw[ct][:, off : off + ln]
            nc.scalar.activation(dslice, pcw[:, :ln], AF.Sigmoid)
            nc.vector.tensor_mul(
                dslice, dslice, w_sb[ct][:, off : off + ln]
            )
    return dynw


def _build(reps: int = 1):
    nc = bacc.Bacc("TRN2", target_bir_lowering=False, debug=False, num_devices=N_CORES)

    x_in = nc.declare_dram_parameter("x", [BL, C, XPADN], BF16, isOutput=False)
    wt_in = nc.declare_dram_parameter("w_t", [C, IJO], BF16, isOutput=False)
    cog4_in = nc.declare_dram_parameter("cog4", [CH, IJO], BF16, isOutput=False)
    id_in = nc.declare_dram_parameter("ident", [128, 128], BF16, isOutput=False)
    fc1_in = nc.declare_dram_parameter("fc1_wt", [C, C], F32, isOutput=False)
    fc2_in = nc.declare_dram_parameter("fc2_wt", [C, CH * C], BF16, isOutput=False)
    fc2b_in = nc.declare_dram_parameter("fc2b", [128, UQ], F32, isOutput=False)
    y_out = nc.declare_dram_parameter("y", [BL, O, H, W], BF16, isOutput=True)

    with tile.TileContext(nc) as tc:
        with (
            tc.tile_pool(name="sbuf", bufs=1) as pool,
            tc.tile_pool(name="psum_fc", bufs=1, space="PSUM") as psum_fc,
            tc.tile_pool(name="psum_cw", bufs=1, space="PSUM") as psum_cw,
            tc.tile_pool(name="psum_kt", bufs=1, space="PSUM") as psum_kt,
            tc.tile_pool(name="psum_cv", bufs=1, space="PSUM") as psum_cv,
        ):
            XQ = XPADN // NXQ

            def load_x(rep, b):
                per_ct = [
                    pool.tile(
                        [128, XPADN], BF16, name=f"x{rep}_{b}_{ct}", tag=f"x{ct}", bufs=3
                    )
                    for ct in range(CT)
                ]
                # interleave ct0/ct1 quarters so both pooled reduces pipeline
                for q in range(NXQ):
                    for ct in range(CT):
                        nc.sync.dma_start(
                            per_ct[ct][:, q * XQ : (q + 1) * XQ],
                            x_in[b, ct * 128 : (ct + 1) * 128, q * XQ : (q + 1) * XQ],
                        )
                return per_ct

            def xview(t):
                return t[:, : HP * WP].rearrange("p (h w) -> p h w", h=HP)

            # prewarm the ACT function tables while the first DMAs stream,
            # and fire one dummy matmul so the PE p-state ramp (3us to full
            # clock from first PE activity) completes before real work
            warm = pool.tile([128, 1], F32, name="warm", tag="warm")
            nc.vector.memset(warm[:], 0.0)
            nc.scalar.activation(warm[:], warm[:], AF.Copy)
            nc.scalar.activation(warm[:], warm[:], AF.Relu)
            nc.scalar.activation(warm[:], warm[:], AF.Sigmoid)
            wp = psum_fc.tile([128, CT], F32, name="wp", tag="pfcA", bufs=1)
            nc.tensor.matmul(wp[:1, :1], warm[:], warm[:], start=True, stop=True,
                             skip_group_check=True)

            # The DMA engine drains its queue serially, so emission order is
            # the priority order: x0 heads the longest dependency chain
            # (pooling), then the params in the order the synth chain
            # consumes them, then x1.
            xsb = [load_x(0, 0)]
            fc1_sb = []
            for jt in range(CT):
                t = pool.tile([128, C], F32, name=f"fc1_sb{jt}", tag=f"fc1_sb{jt}")
                nc.sync.dma_start(t[:], fc1_in[jt * 128 : (jt + 1) * 128, :])
                fc1_sb.append(t)
            fc2b_sb = pool.tile([128, UQ], F32, name="fc2b_sb", tag="fc2b_sb")
            nc.sync.dma_start(fc2b_sb[:], fc2b_in[:])
            fc2_sb = []
            for jt in range(CT):
                t2 = pool.tile([128, CH * C], BF16, name=f"fc2_sb{jt}", tag=f"fc2_sb{jt}")
                nc.sync.dma_start(t2[:], fc2_in[jt * 128 : (jt + 1) * 128, :])
                fc2_sb.append(t2)
            ident_sb = pool.tile([128, 128], BF16, name="ident_sb", tag="ident_sb")
            nc.sync.dma_start(ident_sb[:], id_in[:])
            cog4_sb = pool.tile([CH, IJO], BF16, name="cog4_sb", tag="cog4_sb")
            nc.sync.dma_start(cog4_sb[:], cog4_in[:])
            w_sb = []
            for ct in range(CT):
                t = pool.tile([128, IJO], BF16, name=f"w_sb{ct}", tag=f"w_sb{ct}")
                nc.sync.dma_start(t[:], wt_in[ct * 128 : (ct + 1) * 128, :])
                w_sb.append(t)

            def emit_block(b, ot, r0, nr, xtiles, dynw):
                """One conv row block: rows [r0, r0+nr) of output tile ot,
                ct-major taps, copy to a bf16 stage tile, store."""
                nm = nr * W
                pc = psum_cv.tile(
                    [128, NCONV], F32, name=f"pc{b}_{ot}_{r0}", tag="pc", bufs=3
                )
                mm = 0
                for ct in range(CT):
                    for di in range(KS):
                        for dj in range(KS):
                            lo = (di * KS + dj) * O + ot * 128
                            nc.tensor.matmul(
                                pc[:, :nm],
                                dynw[ct][:, lo : lo + 128],
                                xview(xtiles[ct])[
                                    :, r0 + di : r0 + di + nr, dj : dj + W
                                ],
                                start=(mm == 0),
                                stop=(mm == KS * KS * CT - 1),
                            )
                            mm += 1
                stg = pool.tile(
                    [128, NCONV], BF16, name=f"st{b}_{ot}_{r0}", tag="stage", bufs=4
                )
                nc.vector.tensor_copy(stg[:, :nm], pc[:, :nm])
                nc.sync.dma_start(
                    y_out[b, ot * 128 : (ot + 1) * 128, r0 : r0 + nr, :],
                    stg[:, :nm].rearrange("p (h w) -> p h w", h=nr),
                )

            for rep in range(reps):
                if rep > 0:
                    xsb = [load_x(rep, 0)]

                synth_tiles = (pool, psum_fc, xsb, fc1_sb, fc2_sb, fc2b_sb)
                dynw_tiles = (pool, psum_cw, psum_kt, cog4_sb, ident_sb, w_sb)

                kget0 = _emit_synth(nc, 0, synth_tiles, act_assist=True)
                xsb.append(load_x(rep, 1))
                dynw = _emit_dynw(nc, 0, kget0, dynw_tiles)

                for b in range(BL):
                    kget_next = None
                    if b + 1 < BL:
                        kget_next = _emit_synth(nc, b + 1, synth_tiles)
                        if b + 2 < BL:
                            xsb.append(load_x(rep, b + 2))

                    last = rep == reps - 1 and b == BL - 1
                    dynw_next = None
                    for ot in range(OT):
                        if last and ot == OT - 1:
                            # split the final block 7+1 so the tail's
                            # copy + store is as small as possible
                            blocks = [(r, 8) for r in range(0, H - 8, 8)]
                            blocks += [(H - 8, 7), (H - 1, 1)]
                        else:
                            blocks = [(r, 8) for r in range(0, H, 8)]
                        for r0, nr in blocks:
                            emit_block(b, ot, r0, nr, xsb[b], dynw)
                        if ot == 0 and kget_next is not None:
                            dynw_next = _emit_dynw(nc, b + 1, kget_next, dynw_tiles)
                    if dynw_next is not None:
                        dynw = dynw_next

    nc.compile()
    return nc


def _prep_static(fc1_w, fc2_w, fc2_b, cog_weight, weight):
    w_t = np.ascontiguousarray(weight.transpose(1, 2, 3, 0)).reshape(C, IJO)
    cog4 = np.ascontiguousarray(cog_weight.transpose(1, 2, 3, 0)).reshape(CH, IJO)
    # fc2 column permutation: kern col q = ct*CH + t holds, at partition u,
    # the fc2 output for channel c = ct*128 + u and chunk index t
    fc2_wt = np.ascontiguousarray(fc2_w.T)                       # [C, CH*C]
    fc2_wt2 = np.ascontiguousarray(
        fc2_wt.reshape(C, CT, 128, CH).transpose(0, 1, 3, 2)
    ).reshape(C, CH * C)
    fc2b2 = np.ascontiguousarray(
        fc2_b.reshape(CT, 128, CH).transpose(1, 0, 2)
    ).reshape(128, UQ)
    fc1_wt = np.ascontiguousarray(fc1_w.T) * np.float32(HWINV)
    ident = np.eye(128, dtype=np.float32)
    return dict(
        w_t=w_t.astype(ml_dtypes.bfloat16),
        cog4=cog4.astype(ml_dtypes.bfloat16),
        ident=ident.astype(ml_dtypes.bfloat16),
        fc1_wt=fc1_wt.astype(np.float32),
        fc2_wt=fc2_wt2.astype(ml_dtypes.bfloat16),
        fc2b=fc2b2.astype(np.float32),
    )


def _pad_x(x):
    """[B, C, H, W] -> flat host-padded bf16 [B, C, XPADN] (58x58, zeros)."""
    xp = np.zeros((x.shape[0], C, XPADN), ml_dtypes.bfloat16)
    xp[:, :, : HP * WP].reshape(x.shape[0], C, HP, WP)[
        :, :, 1 : H + 1, 1 : W + 1
    ] = x.astype(ml_dtypes.bfloat16)
    return xp


def kernel(x, fc1_w, fc2_w, fc2_b, cog_weight, weight):
    x = np.asarray(x, dtype=np.float32)
    static = _prep_static(
        np.asarray(fc1_w, np.float32), np.asarray(fc2_w, np.float32),
        np.asarray(fc2_b, np.float32), np.asarray(cog_weight, np.float32),
        np.asarray(weight, np.float32),
    )
    xp = _pad_x(x)
    if "nc" not in _CACHE:
        _CACHE["nc"] = _build()
    nc = _CACHE["nc"]
    in_maps = [dict(x=xp[k * BL : (k + 1) * BL], **static) for k in range(N_CORES)]
    res = run_bass_kernel_spmd(nc, in_maps, core_ids=list(range(N_CORES)))
    return np.concatenate(
        [res.results[k]["y"] for k in range(N_CORES)], axis=0
    ).astype(np.float32)


# revision 10
# speedup vs baseline: 1.0080x; 1.0080x over previous
"""COGConv2d Trainium2 kernel (8 NeuronCores, Bass/Tile).

Reference computation (per sample b):
  pooled = mean_{h,w} x[b]                               [C]
  h      = relu(fc1_w @ pooled)                          [C]
  kern   = fc2_w @ h + fc2_b                             [CH*C], u = c*CH + t
  cw[o,c,i,j]   = sum_t kern[c*CH+t] * cog[o,t,i,j]
  dynw[o,c,i,j] = sigmoid(cw) * weight[o,c,i,j]
  y[b]   = conv2d(x[b], dynw, pad=1)                     [O,H,W]

Sharding: data-parallel over batch B=32 across 8 cores (4 samples/core);
the small static params are replicated to every core.

Per core, the conv runs as 9-tap shifted matmuls accumulating in PSUM
([dynw tap slice].T @ [shifted x window], contraction over channels),
ct-major so each row block only needs dynw for one ctile at a time.
x is zero-padded to 58x58 on the host so every tap window is a simple
AP slice of one SBUF tile.

The whole datapath is bf16 (PE runs bf16 at the same 1 cycle/row as
fp32r, but DMA bytes halve; y is stored bf16 and widened on the host).
The per-sample weight synthesis runs on-chip: all 16 fc2 matmuls
accumulate into one PSUM bank (one column per output chunk) and a single
DVE add applies the bias, with a host-side fc2 column permutation chosen
so a PE transpose of each [128, CH] kern slice directly yields the
[CH, 128] stationary for a compact K=CH cw matmul against cog4[t, :].
The synthesis for sample b+1 is pipelined into sample b's conv. Startup
DMAs are issued in critical-path order (x0, fc1, fc2, ident, cog4, w)
since the DMA engine drains its queue serially, and a dummy matmul right
after init starts the PE p-state ramp so real matmuls run at full clock.
The final row block is split 7+1 so the last store's copy+DMA tail is
minimal.
"""

import numpy as np
import ml_dtypes

import concourse.bacc as bacc
import concourse.mybir as mybir
import concourse.tile as tile
from concourse.bass_utils import run_bass_kernel_spmd

F32 = mybir.dt.float32
BF16 = mybir.dt.bfloat16
AF = mybir.ActivationFunctionType

N_CORES = 8
B, C, O, KS, H, W, CH = 32, 256, 256, 3, 56, 56, 4
BL = B // N_CORES            # samples per core
HW = H * W                   # 3136
HP, WP = H + 2, W + 2        # host-padded spatial (58x58)
XPADN = HP * WP + 4          # padded map + 4 spare cols (3368)
IJO = KS * KS * O            # 2304; dyn-weight free index = (i*3+j)*O + o
CT = C // 128                # contraction tiles (2)
OT = O // 128                # output-channel tiles (2)
RROWS = 8                    # output rows per conv matmul block
RB = H // RROWS              # row blocks (7)
NCONV = RROWS * W            # conv matmul moving size (448)
HWINV = 1.0 / HW
UQ = CH * C // 128           # fc2 output chunks (8)
NXQ = 2                      # x load split for load/reduce overlap
CW_CHUNKS = [(o, min(512, IJO - o)) for o in range(0, IJO, 512)]

_CACHE = {}


def _emit_synth(nc, b, ctx_tiles, act_assist=False):
    """Weight synthesis part 1 for sample b: pooled -> fc1 -> fc2 -> kern.

    kern layout: [128 partitions (u), q cols] where column q = ct*CH + t
    holds, at partition u, the fc2 output for (c = ct*128 + u, t) -- the
    host permutes fc2's columns to produce this order. All fc2 matmuls
    accumulate into single-bank PSUM columns (no per-chunk ACT ping-pong);
    one DVE add applies the bias. Returns kget(ct) -> [128, CH] slice.
    """
    (pool, psum_fc, xsb, fc1_sb, fc2_sb, fc2b_sb) = ctx_tiles

    pooled = [
        pool.tile([128, 1], F32, name=f"pooled{b}_{ct}", tag=f"pooled{ct}", bufs=2)
        for ct in range(CT)
    ]
    xq = XPADN // NXQ
    for ct in range(CT):
        rp = pool.tile([128, NXQ], F32, name=f"rp{b}_{ct}", tag=f"rp{ct}", bufs=2)
        for q in range(NXQ):
            if act_assist and ct == 1:
                # head: DVE is the pooled bottleneck; ct1 partials on ACT
                scr = pool.tile([128, xq], BF16, name=f"rs{b}_{q}", tag="rs", bufs=2)
                nc.scalar.activation(
                    scr[:], xsb[b][ct][:, q * xq : (q + 1) * xq],
                    AF.Copy, accum_out=rp[:, q : q + 1],
                )
            else:
                nc.vector.tensor_reduce(
                    out=rp[:, q : q + 1],
                    in_=xsb[b][ct][:, q * xq : (q + 1) * xq],
                    axis=mybir.AxisListType.X, op=mybir.AluOpType.add,
                )
        nc.vector.tensor_reduce(
            out=pooled[ct][:], in_=rp[:], axis=mybir.AxisListType.X,
            op=mybir.AluOpType.add,
        )

    # pooled is f32 (reduce output precision is gated); a free ap=1 copy
    # narrows it to bf16 so the fc matmuls run on bf16 operands
    poolb = []
    for ct in range(CT):
        t = pool.tile([128, 1], BF16, name=f"poolb{b}_{ct}", tag=f"poolb{ct}", bufs=2)
        nc.vector.tensor_copy(t[:], pooled[ct][:])
        poolb.append(t)

    hvec = [
        pool.tile([128, 1], BF16, name=f"h{b}_{it}", tag=f"h{it}", bufs=2)
        for it in range(CT)
    ]
    for it in range(CT):
        pfc = psum_fc.tile([128, 1], F32, name=f"pfc1_{b}_{it}", tag="pfc", bufs=2)
        for jt in range(CT):
            nc.tensor.matmul(
                pfc[:], fc1_sb[jt][:, it * 128 : (it + 1) * 128], poolb[jt][:],
                start=(jt == 0), stop=(jt == CT - 1),
            )
        nc.scalar.activation(hvec[it][:], pfc[:], AF.Relu)

    kern = pool.tile([128, UQ], BF16, name=f"kern_sb{b}", tag="kern_sb", bufs=2)
    for q in range(UQ):
        pfc = psum_fc.tile([128, 1], F32, name=f"pfc2_{b}_{q}", tag="pfc", bufs=2)
        for jt in range(CT):
            nc.tensor.matmul(
                pfc[:], fc2_sb[jt][:, q * 128 : (q + 1) * 128], hvec[jt][:],
                start=(jt == 0), stop=(jt == CT - 1),
            )
        nc.scalar.activation(
            kern[:, q : q + 1], pfc[:], AF.Identity,
            bias=fc2b_sb[:, q : q + 1],
        )

    return lambda ct: kern[:, ct * CH : (ct + 1) * CH]


def _emit_dynw(nc, b, kget, ctx_tiles):
    """Part 2: transpose kern slice -> K=CH cw matmuls -> sigmoid -> *w.

    cw[c, n] = sum_t kern2[t, c] * cog4[t, n] with kern2 = kget(ct).T via
    the PE transpose path (output bounced through PSUM then SBUF).
    """
    (pool, psum_cw, psum_kt, cog4_sb, ident_sb, w_sb) = ctx_tiles
    dynw = [
        pool.tile([128, IJO], BF16, name=f"dynw{b}_{ct}", tag=f"dynw{ct}", bufs=2)
        for ct in range(CT)
    ]
    for ct in range(CT):
        ktr = psum_kt.tile([CH, 128], BF16, name=f"ktr{b}_{ct}", tag="ktr", bufs=1)
        nc.tensor.transpose(ktr[:], kget(ct), ident_sb[:])
        kern2 = pool.tile([CH, 128], BF16, name=f"k2_{b}_{ct}", tag=f"k2_{ct}", bufs=2)
        nc.vector.tensor_copy(kern2[:], ktr[:])
        for off, ln in CW_CHUNKS:
            pcw = psum_cw.tile(
                [128, 512], F32, name=f"pcw{b}_{ct}_{off}", tag="pcw", bufs=2
            )
            nc.tensor.matmul(
                pcw[:, :ln], kern2[:], cog4_sb[:, off : off + ln],
                start=True, stop=True,
            )
            dslice = dynw[ct][:, off : off + ln]
            nc.scalar.activation(dslice, pcw[:, :ln], AF.Sigmoid)
            nc.vector.tensor_mul(
                dslice, dslice, w_sb[ct][:, off : off + ln]
            )
    return dynw


def _build(reps: int = 1):
    nc = bacc.Bacc("TRN2", target_bir_lowering=False, debug=False, num_devices=N_CORES)

    x_in = nc.declare_dram_parameter("x", [BL, C, XPADN], BF16, isOutput=False)
    wt_in = nc.declare_dram_parameter("w_t", [C, IJO], BF16, isOutput=False)
    cog4_in = nc.declare_dram_parameter("cog4", [CH, IJO], BF16, isOutput=False)
    id_in = nc.declare_dram_parameter("ident", [128, 128], BF16, isOutput=False)
    fc1_in = nc.declare_dram_parameter("fc1_wt", [C, C], BF16, isOutput=False)
    fc2_in = nc.declare_dram_parameter("fc2_wt", [C, CH * C], BF16, isOutput=False)
    fc2b_in = nc.declare_dram_parameter("fc2b", [128, UQ], F32, isOutput=False)
    y_out = nc.declare_dram_parameter("y", [BL, O, H, W], BF16, isOutput=True)

    with tile.TileContext(nc) as tc:
        with (
            tc.tile_pool(name="sbuf", bufs=1) as pool,
            tc.tile_pool(name="psum_fc", bufs=1, space="PSUM") as psum_fc,
            tc.tile_pool(name="psum_cw", bufs=1, space="PSUM") as psum_cw,
            tc.tile_pool(name="psum_kt", bufs=1, space="PSUM") as psum_kt,
            tc.tile_pool(name="psum_cv", bufs=1, space="PSUM") as psum_cv,
        ):
            XQ = XPADN // NXQ

            def load_x(rep, b):
                per_ct = [
                    pool.tile(
                        [128, XPADN], BF16, name=f"x{rep}_{b}_{ct}", tag=f"x{ct}", bufs=3
                    )
                    for ct in range(CT)
                ]
                # interleave ct0/ct1 quarters so both pooled reduces pipeline
                for q in range(NXQ):
                    for ct in range(CT):
                        nc.sync.dma_start(
                            per_ct[ct][:, q * XQ : (q + 1) * XQ],
                            x_in[b, ct * 128 : (ct + 1) * 128, q * XQ : (q + 1) * XQ],
                        )
                return per_ct

            def xview(t):
                return t[:, : HP * WP].rearrange("p (h w) -> p h w", h=HP)

            # prewarm the ACT function tables while the first DMAs stream
            warm = pool.tile([128, 1], F32, name="warm", tag="warm")
            nc.vector.memset(warm[:], 0.0)
            nc.scalar.activation(warm[:], warm[:], AF.Copy)
            nc.scalar.activation(warm[:], warm[:], AF.Relu)
            nc.scalar.activation(warm[:], warm[:], AF.Sigmoid)
            # chain dummy matmuls across the DMA-bound startup window: the PE
            # p-state ramp (3us of near-continuous activity to reach full
            # clock, reset by idles over ~3us) then completes before the
            # first real matmul instead of slowing it down
            warm2 = pool.tile([128, 512], BF16, name="warm2", tag="warm2")
            nc.vector.memset(warm2[:], 0.0)
            for i in range(20):
                wpc = psum_cw.tile([128, 512], F32, name=f"wpc{i}", tag="pcw", bufs=2)
                nc.tensor.matmul(wpc[:1, :], warm2[:, :1], warm2[:],
                                 start=True, stop=True, skip_group_check=True)

            # The DMA engine drains its queue serially, so emission order is
            # the priority order: x0 heads the longest dependency chain
            # (pooling), then the params in the order the synth chain
            # consumes them, then x1.
            xsb = [load_x(0, 0)]
            fc1_sb = []
            for jt in range(CT):
                t = pool.tile([128, C], BF16, name=f"fc1_sb{jt}", tag=f"fc1_sb{jt}")
                nc.sync.dma_start(t[:], fc1_in[jt * 128 : (jt + 1) * 128, :])
                fc1_sb.append(t)
            fc2b_sb = pool.tile([128, UQ], F32, name="fc2b_sb", tag="fc2b_sb")
            nc.sync.dma_start(fc2b_sb[:], fc2b_in[:])
            fc2_sb = []
            for jt in range(CT):
                t2 = pool.tile([128, CH * C], BF16, name=f"fc2_sb{jt}", tag=f"fc2_sb{jt}")
                nc.sync.dma_start(t2[:], fc2_in[jt * 128 : (jt + 1) * 128, :])
                fc2_sb.append(t2)
            ident_sb = pool.tile([128, 128], BF16, name="ident_sb", tag="ident_sb")
            nc.sync.dma_start(ident_sb[:], id_in[:])
            cog4_sb = pool.tile([CH, IJO], BF16, name="cog4_sb", tag="cog4_sb")
            nc.sync.dma_start(cog4_sb[:], cog4_in[:])
            w_sb = []
            for ct in range(CT):
                t = pool.tile([128, IJO], BF16, name=f"w_sb{ct}", tag=f"w_sb{ct}")
                nc.sync.dma_start(t[:], wt_in[ct * 128 : (ct + 1) * 128, :])
                w_sb.append(t)

            def emit_block(b, ot, r0, nr, xtiles, dynw):
                """One conv row block: rows [r0, r0+nr) of output tile ot,
                ct-major taps, copy to a bf16 stage tile, store."""
                nm = nr * W
                pc = psum_cv.tile(
                    [128, NCONV], F32, name=f"pc{b}_{ot}_{r0}", tag="pc", bufs=3
                )
                mm = 0
                for ct in range(CT):
                    for di in range(KS):
                        for dj in range(KS):
                            lo = (di * KS + dj) * O + ot * 128
                            nc.tensor.matmul(
                                pc[:, :nm],
                                dynw[ct][:, lo : lo + 128],
                                xview(xtiles[ct])[
                                    :, r0 + di : r0 + di + nr, dj : dj + W
                                ],
                                start=(mm == 0),
                                stop=(mm == KS * KS * CT - 1),
                            )
                            mm += 1
                stg = pool.tile(
                    [128, NCONV], BF16, name=f"st{b}_{ot}_{r0}", tag="stage", bufs=4
                )
                nc.vector.tensor_copy(stg[:, :nm], pc[:, :nm])
                nc.sync.dma_start(
                    y_out[b, ot * 128 : (ot + 1) * 128, r0 : r0 + nr, :],
                    stg[:, :nm].rearrange("p (h w) -> p h w", h=nr),
                )

            for rep in range(reps):
                if rep > 0:
                    xsb = [load_x(rep, 0)]

                synth_tiles = (pool, psum_fc, xsb, fc1_sb, fc2_sb, fc2b_sb)
                dynw_tiles = (pool, psum_cw, psum_kt, cog4_sb, ident_sb, w_sb)

                kget0 = _emit_synth(nc, 0, synth_tiles, act_assist=True)
                xsb.append(load_x(rep, 1))
                dynw = _emit_dynw(nc, 0, kget0, dynw_tiles)

                for b in range(BL):
                    kget_next = None
                    if b + 1 < BL:
                        kget_next = _emit_synth(nc, b + 1, synth_tiles)
                        if b + 2 < BL:
                            xsb.append(load_x(rep, b + 2))

                    last = rep == reps - 1 and b == BL - 1
                    dynw_next = None
                    for ot in range(OT):
                        if last and ot == OT - 1:
                            # split the final block 6+2 so the tail's
                            # copy + store is small (one final HWDGE issue)
                            blocks = [(r, 8) for r in range(0, H - 8, 8)]
                            blocks += [(H - 8, 6), (H - 2, 2)]
                        else:
                            blocks = [(r, 8) for r in range(0, H, 8)]
                        for r0, nr in blocks:
                            emit_block(b, ot, r0, nr, xsb[b], dynw)
                        if ot == 0 and kget_next is not None:
                            dynw_next = _emit_dynw(nc, b + 1, kget_next, dynw_tiles)
                    if dynw_next is not None:
                        dynw = dynw_next

    nc.compile()
    return nc


def _prep_static(fc1_w, fc2_w, fc2_b, cog_weight, weight):
    w_t = np.ascontiguousarray(weight.transpose(1, 2, 3, 0)).reshape(C, IJO)
    cog4 = np.ascontiguousarray(cog_weight.transpose(1, 2, 3, 0)).reshape(CH, IJO)
    # fc2 column permutation: kern col q = ct*CH + t holds, at partition u,
    # the fc2 output for channel c = ct*128 + u and chunk index t
    fc2_wt = np.ascontiguousarray(fc2_w.T)                       # [C, CH*C]
    fc2_wt2 = np.ascontiguousarray(
        fc2_wt.reshape(C, CT, 128, CH).transpose(0, 1, 3, 2)
    ).reshape(C, CH * C)
    fc2b2 = np.ascontiguousarray(
        fc2_b.reshape(CT, 128, CH).transpose(1, 0, 2)
    ).reshape(128, UQ)
    fc1_wt = np.ascontiguousarray(fc1_w.T) * np.float32(HWINV)
    ident = np.eye(128, dtype=np.float32)
    return dict(
        w_t=w_t.astype(ml_dtypes.bfloat16),
        cog4=cog4.astype(ml_dtypes.bfloat16),
        ident=ident.astype(ml_dtypes.bfloat16),
        fc1_wt=fc1_wt.astype(ml_dtypes.bfloat16),
        fc2_wt=fc2_wt2.astype(ml_dtypes.bfloat16),
        fc2b=fc2b2.astype(np.float32),
    )


def _pad_x(x):
    """[B, C, H, W] -> flat host-padded bf16 [B, C, XPADN] (58x58, zeros)."""
    xp = np.zeros((x.shape[0], C, XPADN), ml_dtypes.bfloat16)
    xp[:, :, : HP * WP].reshape(x.shape[0], C, HP, WP)[
        :, :, 1 : H + 1, 1 : W + 1
    ] = x.astype(ml_dtypes.bfloat16)
    return xp


def kernel(x, fc1_w, fc2_w, fc2_b, cog_weight, weight):
    x = np.asarray(x, dtype=np.float32)
    static = _prep_static(
        np.asarray(fc1_w, np.float32), np.asarray(fc2_w, np.float32),
        np.asarray(fc2_b, np.float32), np.asarray(cog_weight, np.float32),
        np.asarray(weight, np.float32),
    )
    xp = _pad_x(x)
    if "nc" not in _CACHE:
        _CACHE["nc"] = _build()
    nc = _CACHE["nc"]
    in_maps = [dict(x=xp[k * BL : (k + 1) * BL], **static) for k in range(N_CORES)]
    res = run_bass_kernel_spmd(nc, in_maps, core_ids=list(range(N_CORES)))
    return np.concatenate(
        [res.results[k]["y"] for k in range(N_CORES)], axis=0
    ).astype(np.float32)
